# revision 1
# baseline (speedup 1.0000x reference)
"""Single-head self-attention (B=4, S=2048, D=1024, fp32) on 8 trn2 NeuronCores.

Sharding: each core owns (batch b = core//2, sequence half h = core%2).
V is computed only for the core's own 1024 sequence rows (j-split, no
duplication). Q and K are never materialized: expanding
  scores[q,j] = (x_q Wq + bq).(x_j Wk + bk)
             = x_q.G.x_j + x_j.w + (x_q.u + bq.bk)
with G = Wq Wk^T, w = Wk bq, u = Wq bk (host-precomputed weight fusion,
like BN folding). The x_q.u term is constant per query row and softmax-
invariant, so it is dropped; x_j.w + bq.bk folds into the exp's
per-partition bias. The device computes M = G^T-contraction @ x^T
(own-j sized) then scores^T = M-contraction @ x^T.
Each core then produces the *partial* softmax numerator
  pre[q, :] = sum_{j in own half} exp(q.k_j/sqrt(D)) * (v_j + bv)
and the partial denominator den[q]; the host combines the halves exactly:
  out = (pre_h0 + pre_h1) / (den_h0 + den_h1)
(no max-subtraction needed: scores ~ N(0,1), fp32 exp cannot overflow).

Everything is laid out "transposed" ([d, row]) so the contraction dim is
always on SBUF partitions and no on-chip transposes are ever needed:
  qT/kT = W.T @ x.T come from lhsT=W (native), rhs=xT (host-pretransposed)
  v (native [row, d]) comes from lhsT=xT chunk, rhs=Wv (native)
The ones column appended to V yields den in the same PSUM accumulation.
Matmuls run in fp16 (1 PE cycle/row like bf16, but 3 more mantissa bits;
all values are far from fp16 range limits) with fp32 PSUM accumulation.
"""

import numpy as np
import ml_dtypes

import concourse.bass as bass
import concourse.mybir as mybir
import concourse.tile as tile
from concourse.bass_utils import run_bass_kernel_spmd

F16 = mybir.dt.float16
F32 = mybir.dt.float32
AFT = mybir.ActivationFunctionType

B, S, D = 4, 2048, 1024
NCORES = 8
P = 128
DC = D // P            # 8 contraction chunks
JROWS = S // 2         # 1024 own k/v rows per core
JC = JROWS // P        # 8 own j chunks
QB = S // 512          # 4 query col-blocks of 512 (all rows of the batch)
SCALE = 1.0 / np.sqrt(np.float32(D))  # 1/32

_CACHED = {}


def _split_excess_waits(nc, max_waits=1):
    """walrus in this env rejects >1 sync-wait per instruction (Drain at Tile
    exit carries one per live semaphore); move extras onto same-engine NOPs."""
    for f in nc.m.functions:
        for bb in f.blocks:
            new_list, changed = [], False
            for ins in bb.instructions:
                si = getattr(ins, "sync_info", None)
                ow = list(si.on_wait) if si and si.on_wait else []
                if len(ow) > max_waits:
                    extra, keep = ow[:-max_waits], ow[-max_waits:]
                    for k, w in enumerate(extra):
                        new_list.append(
                            mybir.InstNoOp(
                                name=f"{ins.name}_ws{k}",
                                engine=ins.engine,
                                sync_info=mybir.SyncInfo(on_wait=[w], on_update=[]),
                                bass_nofuse=True,
                            )
                        )
                    si.on_wait = keep
                    changed = True
                new_list.append(ins)
            if changed:
                bb.instructions = new_list


def _build():
    nc = bass.Bass("TRN2", target_bir_lowering=False, debug=False, num_devices=NCORES)

    # xT is the whole batch transposed, own j-half first (host permutes).
    xT_d = nc.dram_tensor("xT", [D, S], F16, kind="ExternalInput").ap()
    gT_d = nc.dram_tensor("gT", [D, D], F16, kind="ExternalInput").ap()
    wv_d = nc.dram_tensor("wv", [D, D], F16, kind="ExternalInput").ap()
    w_d = nc.dram_tensor("w", [P, DC], F16, kind="ExternalInput").ap()
    c0s_d = nc.dram_tensor("c0s", [P, 1], F32, kind="ExternalInput").ap()
    bv_d = nc.dram_tensor("bv", [1, D], F32, kind="ExternalInput").ap()
    pre_d = nc.dram_tensor("pre", [S, D], F32, kind="ExternalOutput").ap()
    den_d = nc.dram_tensor("den", [S, 1], F32, kind="ExternalOutput").ap()

    with tile.TileContext(nc) as tc:
        with (
            tc.tile_pool(name="persist", bufs=1) as persist,
            tc.tile_pool(name="outp", bufs=3) as outp,
            tc.tile_pool(name="small", bufs=8) as small,
        ):
            # ---- persistent SBUF ----
            xT_sb = persist.tile([P, DC, S], F16, tag="xT")
            M_sb = persist.tile([P, DC, JROWS], F16, tag="M")
            v_sb = persist.tile([P, JC, D], F16, tag="v")
            bqk_sb = persist.tile([P, JC], F32, tag="bqk")
            w_sb = persist.tile([P, DC], F16, tag="w")
            c0s_sb = persist.tile([P, 1], F32, tag="c0s")
            bv_sb = persist.tile([P, D], F32, tag="bv")
            ones_sb = persist.tile([P, 1], F16, tag="ones")

            nc.vector.memset(ones_sb, 1.0)

            # PE warmup: throwaway matmuls during the initial DMA
            # wait so the HAM clock gate reaches full rate (and the cost
            # model's p-state ramp expires) before real work arrives.
            warm_sb = persist.tile([P, 512], F16, tag="warm")
            nc.vector.memset(warm_sb, 0.0)
            with tc.tile_pool(name="psW", bufs=1, space="PSUM") as psW:
                pw = psW.tile([P, 512], F32, tag="psW")
                for _ in range(8):
                    nc.tensor.matmul(
                        pw, warm_sb[:, 0:P], warm_sb, start=True, stop=True
                    )

            # ---- phase A: projections ----
            with (
                tc.tile_pool(name="pA_in", bufs=1) as pin,
                tc.tile_pool(name="psA", bufs=6, space="PSUM") as psA,
            ):
                gT_sb = pin.tile([P, DC, D], F16, tag="gT")
                wv_sb = pin.tile([P, DC, D], F16, tag="wv")
                # DMA order == consumption order: (gT, xT own half) feed M
                # which runs first; then wv for V, then the rest of xT.
                for c in range(DC):
                    cs = slice(c * P, (c + 1) * P)
                    nc.sync.dma_start(out=gT_sb[:, c, :], in_=gT_d[cs, :])
                    nc.sync.dma_start(
                        out=xT_sb[:, c, 0:JROWS], in_=xT_d[cs, 0:JROWS]
                    )
                nc.sync.dma_start(out=w_sb, in_=w_d[:, :])
                nc.sync.dma_start(out=c0s_sb, in_=c0s_d[:, :])
                bv_bcast = bass.AP(
                    tensor=bv_d.tensor, offset=bv_d.offset,
                    ap=[[0, P], bv_d.ap[1]],
                )
                nc.gpsimd.dma_start(out=bv_sb, in_=bv_bcast)
                for c in range(DC):
                    cs = slice(c * P, (c + 1) * P)
                    nc.sync.dma_start(out=wv_sb[:, c, :], in_=wv_d[cs, :])
                for c in range(DC):
                    cs = slice(c * P, (c + 1) * P)
                    nc.sync.dma_start(
                        out=xT_sb[:, c, JROWS:S], in_=xT_d[cs, JROWS:S]
                    )

                # M[d, j] = sum_d' G[d,d'] x[j,d']  (own j rows)
                for jb in range(JROWS // 512):
                    rs = slice(jb * 512, (jb + 1) * 512)
                    for m in range(DC):
                        ps = psA.tile([P, 512], F32, tag="psA")
                        for c in range(DC):
                            nc.tensor.matmul(
                                ps,
                                gT_sb[:, c, m * P : (m + 1) * P],
                                xT_sb[:, c, rs],
                                start=(c == 0),
                                stop=(c == DC - 1),
                            )
                        nc.vector.tensor_copy(M_sb[:, m, rs], ps)
                # bias[j] = (x_j.w + bq.bk) / sqrt(D), per-partition [j, 1]
                for j in range(JC):
                    pb = psA.tile([P, 1], F32, tag="psBQ", bufs=2)
                    for c in range(DC):
                        nc.tensor.matmul(
                            pb,
                            xT_sb[:, c, j * P : (j + 1) * P],
                            w_sb[:, c : c + 1],
                            start=(c == 0),
                            stop=(c == DC - 1),
                        )
                    nc.vector.tensor_scalar(
                        out=bqk_sb[:, j : j + 1], in0=pb,
                        scalar1=float(SCALE), scalar2=c0s_sb[:, 0:1],
                        op0=mybir.AluOpType.mult, op1=mybir.AluOpType.add,
                    )
                # v (+bv): own j rows only
                for j in range(JC):
                    for ob in range(2):
                        os_ = slice(ob * 512, (ob + 1) * 512)
                        ps = psA.tile([P, 512], F32, tag="psA")
                        for c in range(DC):
                            nc.tensor.matmul(
                                ps,
                                xT_sb[:, c, j * P : (j + 1) * P],
                                wv_sb[:, c, os_],
                                start=(c == 0),
                                stop=(c == DC - 1),
                            )
                        nc.vector.tensor_add(v_sb[:, j, os_], ps, bv_sb[:, os_])


            # ---- phases B+C per query block ----
            with (
                tc.tile_pool(name="attn", bufs=1) as attnp,
                tc.tile_pool(name="psB", bufs=2, space="PSUM") as psB,
                tc.tile_pool(name="psC", bufs=4, space="PSUM") as psC,
                tc.tile_pool(name="psD", bufs=2, space="PSUM") as psD,
            ):
                for qb in range(QB):
                    qs = slice(qb * 512, (qb + 1) * 512)
                    aT = attnp.tile([P, JC, 512], F16, tag=f"attnT{qb}")
                    # B: scores^T[j, q] = sum_d x[q,d] M[d,j] (+ bqk[j]), exp
                    for j in range(JC):
                        ps = psB.tile([P, 512], F32, tag="psB")
                        for c in range(DC):
                            nc.tensor.matmul(
                                ps,
                                M_sb[:, c, j * P : (j + 1) * P],
                                xT_sb[:, c, qs],
                                start=(c == 0),
                                stop=(c == DC - 1),
                            )
                        nc.scalar.activation(
                            out=aT[:, j, :], in_=ps, func=AFT.Exp,
                            scale=float(SCALE), bias=bqk_sb[:, j : j + 1],
                        )
                    # C: pre = attn^T.T @ [V | 1] (partial over own j)
                    for qc in range(4):
                        qls = slice(qc * P, (qc + 1) * P)
                        po0 = psC.tile([P, 512], F32, tag="psO")
                        po1 = psC.tile([P, 512], F32, tag="psO")
                        pd = psD.tile([P, 1], F32, tag="psD")
                        for j in range(JC):
                            lhs = aT[:, j, qls]
                            st, sp = (j == 0), (j == JC - 1)
                            nc.tensor.matmul(po0, lhs, v_sb[:, j, 0:512], start=st, stop=sp)
                            nc.tensor.matmul(po1, lhs, v_sb[:, j, 512:1024], start=st, stop=sp)
                            nc.tensor.matmul(pd, lhs, ones_sb[:, 0:1], start=st, stop=sp)
                        qrow = qb * 512 + qc * P
                        od = small.tile([P, 1], F32, tag="oden")
                        nc.vector.tensor_copy(od, pd)
                        nc.sync.dma_start(out=den_d[qrow : qrow + P, 0:1], in_=od)
                        for ob, po in ((0, po0), (1, po1)):
                            os_ = slice(ob * 512, (ob + 1) * 512)
                            o = outp.tile([P, 512], F32, tag="o")
                            nc.vector.tensor_copy(o, po)
                            nc.sync.dma_start(
                                out=pre_d[qrow : qrow + P, os_], in_=o
                            )

    _split_excess_waits(nc)
    return nc


def _get_nc():
    if "nc" not in _CACHED:
        _CACHED["nc"] = _build()
    return _CACHED["nc"]


def kernel(x, Wq, bq, Wk, bk, Wv, bv):
    x = np.asarray(x, dtype=np.float32)
    bf = np.float16
    Wq32 = np.asarray(Wq, np.float32)
    Wk32 = np.asarray(Wk, np.float32)
    bq32 = np.asarray(bq, np.float32)
    bk32 = np.asarray(bk, np.float32)
    # weight fusion: G^T = Wk Wq^T so scores = x G x^T; w = Wk bq; c0 = bq.bk
    gT_b = np.ascontiguousarray(Wk32 @ Wq32.T).astype(bf)
    w_t = np.ascontiguousarray((Wk32 @ bq32).reshape(DC, P).T).astype(bf)
    c0s_r = np.full((P, 1), float(SCALE) * float(bq32 @ bk32), np.float32)
    wv_b = np.ascontiguousarray(np.asarray(Wv, np.float32)).astype(bf)
    bv_r = np.ascontiguousarray(np.asarray(bv, np.float32).reshape(1, D))

    in_maps = []
    for core in range(NCORES):
        b, h = core // 2, core % 2
        # own j rows first (j order is internal; q order is undone on gather)
        xb = np.roll(x[b], -h * JROWS, axis=0) if h else x[b]
        xT = np.ascontiguousarray(xb.T).astype(bf)  # [D, S]
        in_maps.append(
            {
                "xT": xT,
                "gT": gT_b,
                "wv": wv_b,
                "w": w_t,
                "c0s": c0s_r,
                "bv": bv_r,
            }
        )

    res = run_bass_kernel_spmd(_get_nc(), in_maps, list(range(NCORES)))
    out = np.empty((B, S, D), np.float32)
    for b in range(B):
        r0, r1 = res.results[2 * b], res.results[2 * b + 1]
        pre = r0["pre"] + np.roll(r1["pre"], JROWS, axis=0)
        den = r0["den"] + np.roll(r1["den"], JROWS, axis=0)
        out[b] = pre / den
    return out



# revision 2
# speedup vs baseline: 1.0065x; 1.0065x over previous
"""Single-head self-attention (B=4, S=2048, D=1024, fp32) on 8 trn2 NeuronCores.

fp8-DoubleRow kernel.
Changes vs v3:
  - one top-level scope for every tile pool, with a single 8-bank PSUM pool
    (7 rotating [128,512] banks + 1 [128,1] bank) shared by all phases:
    pool-exit drains were costing a ~1.8us all-engine barrier at the A->B
    transition plus re-ramp.
  - input DMA halves ordered exactly in M-phase consumption order
    (gh/xh/gl/xl half 1, then half 2, ...).
  - den DMAs issued from the Act queue, pre from SP (spreads DGE work).
"""

import numpy as np
import ml_dtypes

import concourse.bass as bass
import concourse.mybir as mybir
import concourse.tile as tile
from concourse.bass_utils import run_bass_kernel_spmd

F8 = mybir.dt.float8e4
F16 = mybir.dt.float16
F32 = mybir.dt.float32
AFT = mybir.ActivationFunctionType
DR = mybir.MatmulPerfMode.DoubleRow
NP8 = ml_dtypes.float8_e4m3

B, S, D = 4, 2048, 1024
NCORES = 8
P = 128
DC = D // P            # 8 contraction chunks
JROWS = S // 2         # 1024 own k/v rows per core
JC = JROWS // P        # 8 own j chunks
QB = S // 512          # 4 query col-blocks of 512
SCALE = 1.0 / np.sqrt(np.float32(D))  # 1/32
GS = 16.0              # host pre-scale on G
VS = 16.0              # host pre-scale on Wv
WS = 32.0              # host pre-scale on w = Wk bq
CSH = 2.0              # exp shift: a = exp(s - CSH)

_CACHED = {}


def _split_excess_waits(nc, max_waits=1):
    """walrus in this env rejects >1 sync-wait per instruction (Drain at Tile
    exit carries one per live semaphore); move extras onto same-engine NOPs."""
    for f in nc.m.functions:
        for bb in f.blocks:
            new_list, changed = [], False
            for ins in bb.instructions:
                si = getattr(ins, "sync_info", None)
                ow = list(si.on_wait) if si and si.on_wait else []
                if len(ow) > max_waits:
                    extra, keep = ow[:-max_waits], ow[-max_waits:]
                    for k, w in enumerate(extra):
                        new_list.append(
                            mybir.InstNoOp(
                                name=f"{ins.name}_ws{k}",
                                engine=ins.engine,
                                sync_info=mybir.SyncInfo(on_wait=[w], on_update=[]),
                                bass_nofuse=True,
                            )
                        )
                    si.on_wait = keep
                    changed = True
                new_list.append(ins)
            if changed:
                bb.instructions = new_list


def _dram_chunked(dram_ap, col_lo, col_hi, ncols_total):
    """AP over a [D, ncols_total] DRAM tensor reading cols [col_lo, col_hi)
    of all DC chunks, shaped [P, DC, col_hi-col_lo] to match an SBUF tile."""
    n = col_hi - col_lo
    return bass.AP(
        tensor=dram_ap.tensor,
        offset=dram_ap.offset + col_lo,
        ap=[[ncols_total, P], [P * ncols_total, DC], [1, n]],
    )


def _build():
    nc = bass.Bass("TRN2", target_bir_lowering=False, debug=False, num_devices=NCORES)

    xh_d = nc.dram_tensor("xh", [D, S], F8, kind="ExternalInput").ap()
    xl_d = nc.dram_tensor("xl", [D, S], F8, kind="ExternalInput").ap()
    gh_d = nc.dram_tensor("gh", [D, D], F8, kind="ExternalInput").ap()
    gl_d = nc.dram_tensor("gl", [D, D], F8, kind="ExternalInput").ap()
    wvh_d = nc.dram_tensor("wvh", [D, D], F8, kind="ExternalInput").ap()
    wvl_d = nc.dram_tensor("wvl", [D, D], F8, kind="ExternalInput").ap()
    w_d = nc.dram_tensor("w", [P, DC], F8, kind="ExternalInput").ap()
    c0s_d = nc.dram_tensor("c0s", [P, 1], F32, kind="ExternalInput").ap()
    pre_d = nc.dram_tensor("pre", [S, D], F16, kind="ExternalOutput").ap()
    den_d = nc.dram_tensor("den", [S, 1], F32, kind="ExternalOutput").ap()

    with tile.TileContext(nc) as tc:
        with (
            tc.tile_pool(name="persist", bufs=1) as persist,
            tc.tile_pool(name="outp", bufs=4) as outp,
            tc.tile_pool(name="small", bufs=8) as small,
            tc.tile_pool(name="pA_in", bufs=1) as pin,
            tc.tile_pool(name="attn", bufs=1) as attnp,
            tc.tile_pool(name="aw", bufs=4) as awp,
            tc.tile_pool(name="psP", bufs=7, space="PSUM") as psP,
            tc.tile_pool(name="psS", bufs=1, space="PSUM") as psS,
        ):
            # ---- persistent SBUF ----
            xh_sb = persist.tile([P, DC, S], F8, tag="xh")
            xl_sb = persist.tile([P, DC, S], F8, tag="xl")
            Mh_sb = persist.tile([P, DC, JROWS], F8, tag="Mh")
            Ml_sb = persist.tile([P, DC, JROWS], F8, tag="Ml")
            vh_sb = persist.tile([P, JC, D], F8, tag="vh")
            vl_sb = persist.tile([P, JC, D], F8, tag="vl")
            bqk_sb = persist.tile([P, JC], F32, tag="bqk")
            w_sb = persist.tile([P, DC, 1], F8, tag="w")
            c0s_sb = persist.tile([P, 1], F32, tag="c0s")
            ones_sb = persist.tile([P, 2, 1], F8, tag="ones")

            nc.vector.memset(ones_sb, 1.0)

            # PE warmup: throwaway matmuls during the initial DMA wait (the
            # p-state ramp needs ~3us of continuous PE activity).
            warm_sb = persist.tile([P, 512], F16, tag="warm")
            nc.vector.memset(warm_sb, 0.0)
            for _ in range(8):
                pw = psP.tile([P, 512], F32, tag="ps", name="pw")
                nc.tensor.matmul(
                    pw, warm_sb[:, 0:P], warm_sb, start=True, stop=True
                )

            # ---- phase A: M, bias, V ----
            if True:
                gh_sb = pin.tile([P, DC, D], F8, tag="gh")
                gl_sb = pin.tile([P, DC, D], F8, tag="gl")
                wvh_sb = pin.tile([P, DC, D], F8, tag="wvh")
                wvl_sb = pin.tile([P, DC, D], F8, tag="wvl")
                # Batched DMAs in consumption order. M tile (jb, m) needs
                # gh[:, :, m-cols] and xh[:, :, jb-cols]: load col-halves.
                for half in range(2):
                    hs, he = half * 512, (half + 1) * 512
                    nc.sync.dma_start(
                        out=gh_sb[:, :, hs:he], in_=_dram_chunked(gh_d, hs, he, D)
                    )
                    nc.sync.dma_start(
                        out=xh_sb[:, :, hs:he], in_=_dram_chunked(xh_d, hs, he, S)
                    )
                    nc.sync.dma_start(
                        out=gl_sb[:, :, hs:he], in_=_dram_chunked(gl_d, hs, he, D)
                    )
                    nc.sync.dma_start(
                        out=xl_sb[:, :, hs:he], in_=_dram_chunked(xl_d, hs, he, S)
                    )
                nc.sync.dma_start(out=w_sb[:, :, 0], in_=w_d[:, :])
                nc.sync.dma_start(out=c0s_sb, in_=c0s_d[:, :])
                for half in range(2):
                    hs, he = half * 512, (half + 1) * 512
                    nc.sync.dma_start(
                        out=wvh_sb[:, :, hs:he], in_=_dram_chunked(wvh_d, hs, he, D)
                    )
                    nc.sync.dma_start(
                        out=wvl_sb[:, :, hs:he], in_=_dram_chunked(wvl_d, hs, he, D)
                    )
                for half in range(2):
                    hs, he = JROWS + half * 512, JROWS + (half + 1) * 512
                    nc.sync.dma_start(
                        out=xh_sb[:, :, hs:he], in_=_dram_chunked(xh_d, hs, he, S)
                    )
                    nc.sync.dma_start(
                        out=xl_sb[:, :, hs:he], in_=_dram_chunked(xl_d, hs, he, S)
                    )

                # M[d, j] = sum_d' G[d,d'] x[j,d']  (own j rows), 3-term fp8
                for jb in range(JROWS // 512):
                    rs = slice(jb * 512, (jb + 1) * 512)
                    for m in range(DC):
                        ms = slice(m * P, (m + 1) * P)
                        ps = psP.tile([P, 512], F32, tag="ps", name="psa")
                        nmm = 0
                        for lhs_t, rhs_t in (
                            (gh_sb, xh_sb), (gl_sb, xh_sb), (gh_sb, xl_sb)
                        ):
                            for c in range(0, DC, 2):
                                nc.tensor.matmul(
                                    ps,
                                    lhs_t[:, c : c + 2, ms],
                                    rhs_t[:, c : c + 2, rs],
                                    start=(nmm == 0), stop=(nmm == 11),
                                    perf_mode=DR,
                                )
                                nmm += 1
                        nc.scalar.activation(
                            out=Mh_sb[:, m, rs], in_=ps, func=AFT.Copy
                        )
                        nc.vector.tensor_sub(Ml_sb[:, m, rs], ps, Mh_sb[:, m, rs])
                # bias: psum = WS*(x_j.w); bqk = psum*(SCALE/WS) + (SCALE*c0-CSH)
                for j in range(JC):
                    js = slice(j * P, (j + 1) * P)
                    pb = psS.tile([P, 1], F32, tag="sm", name="pb")
                    for c in range(0, DC, 2):
                        nc.tensor.matmul(
                            pb,
                            xh_sb[:, c : c + 2, js],
                            w_sb[:, c : c + 2, 0:1],
                            start=(c == 0), stop=(c == DC - 2),
                            perf_mode=DR,
                        )
                    nc.vector.tensor_scalar(
                        out=bqk_sb[:, j : j + 1], in0=pb,
                        scalar1=float(SCALE / WS), scalar2=c0s_sb[:, 0:1],
                        op0=mybir.AluOpType.mult, op1=mybir.AluOpType.add,
                    )
                # v = x @ Wv (own j rows), 3-term fp8
                for j in range(JC):
                    js = slice(j * P, (j + 1) * P)
                    for ob in range(2):
                        os_ = slice(ob * 512, (ob + 1) * 512)
                        ps = psP.tile([P, 512], F32, tag="ps", name="psa")
                        nmm = 0
                        for lhs_t, rhs_t in (
                            (xh_sb, wvh_sb), (xh_sb, wvl_sb), (xl_sb, wvh_sb)
                        ):
                            for c in range(0, DC, 2):
                                nc.tensor.matmul(
                                    ps,
                                    lhs_t[:, c : c + 2, js],
                                    rhs_t[:, c : c + 2, os_],
                                    start=(nmm == 0), stop=(nmm == 11),
                                    perf_mode=DR,
                                )
                                nmm += 1
                        nc.scalar.activation(
                            out=vh_sb[:, j, os_], in_=ps, func=AFT.Copy
                        )
                        nc.vector.tensor_sub(vl_sb[:, j, os_], ps, vh_sb[:, j, os_])

            # ---- phases B+C, software-pipelined over query blocks ----
            if True:
                ah_t = [
                    attnp.tile([P, JC, 512], F8, tag=f"ah{qb}", name=f"ah{qb}")
                    for qb in range(QB)
                ]
                al_t = [
                    attnp.tile([P, JC, 512], F8, tag=f"al{qb}", name=f"al{qb}")
                    for qb in range(QB)
                ]

                def phase_b(qb):
                    qs = slice(qb * 512, (qb + 1) * 512)
                    for j in range(JC):
                        js = slice(j * P, (j + 1) * P)
                        ps = psP.tile([P, 512], F32, tag="ps", name="psb")
                        nmm = 0
                        for lhs_t, rhs_t in (
                            (Mh_sb, xh_sb), (Ml_sb, xh_sb), (Mh_sb, xl_sb)
                        ):
                            for c in range(0, DC, 2):
                                nc.tensor.matmul(
                                    ps,
                                    lhs_t[:, c : c + 2, js],
                                    rhs_t[:, c : c + 2, qs],
                                    start=(nmm == 0), stop=(nmm == 11),
                                    perf_mode=DR,
                                )
                                nmm += 1
                        a32 = awp.tile([P, 512], F32, tag="a32")
                        nc.scalar.activation(
                            out=a32, in_=ps, func=AFT.Exp,
                            scale=float(SCALE / GS), bias=bqk_sb[:, j : j + 1],
                        )
                        # spread the hi/lo quantize: Pool handles half the j's
                        # alone, Act+DVE split the other half.
                        if j % 2 == 0:
                            nc.gpsimd.tensor_copy(ah_t[qb][:, j, :], a32)
                            nc.gpsimd.tensor_sub(
                                al_t[qb][:, j, :], a32, ah_t[qb][:, j, :]
                            )
                        else:
                            nc.scalar.activation(
                                out=ah_t[qb][:, j, :], in_=a32, func=AFT.Copy
                            )
                            nc.vector.tensor_sub(
                                al_t[qb][:, j, :], a32, ah_t[qb][:, j, :]
                            )

                def phase_c(qb):
                    ah, al = ah_t[qb], al_t[qb]
                    for qc in range(4):
                        qls = slice(qc * P, (qc + 1) * P)
                        po0 = psP.tile([P, 512], F32, tag="ps", name="po0")
                        po1 = psP.tile([P, 512], F32, tag="ps", name="po1")
                        nmm = 0
                        for lhs_t, rhs_t in (
                            (ah, vh_sb), (ah, vl_sb), (al, vh_sb)
                        ):
                            for j in range(0, JC, 2):
                                lhs = lhs_t[:, j : j + 2, qls]
                                st, sp = (nmm == 0), (nmm == 11)
                                nc.tensor.matmul(
                                    po0, lhs, rhs_t[:, j : j + 2, 0:512],
                                    start=st, stop=sp, perf_mode=DR,
                                )
                                nc.tensor.matmul(
                                    po1, lhs, rhs_t[:, j : j + 2, 512:1024],
                                    start=st, stop=sp, perf_mode=DR,
                                )
                                nmm += 1
                        # den[q] = sum_j (ah+al) via ones as moving operand
                        pd = psS.tile([P, 1], F32, tag="sm", name="pd")
                        nmm = 0
                        for a_t in (ah, al):
                            for j in range(0, JC, 2):
                                nc.tensor.matmul(
                                    pd, a_t[:, j : j + 2, qls],
                                    ones_sb[:, 0:2, 0:1],
                                    start=(nmm == 0), stop=(nmm == 7),
                                    perf_mode=DR,
                                )
                                nmm += 1
                        qrow = qb * 512 + qc * P
                        od = small.tile([P, 1], F32, tag="oden")
                        nc.vector.tensor_copy(od, pd)
                        nc.scalar.dma_start(
                            out=den_d[qrow : qrow + P, 0:1], in_=od
                        )
                        for ob, po in ((0, po0), (1, po1)):
                            os_ = slice(ob * 512, (ob + 1) * 512)
                            o = outp.tile([P, 512], F16, tag="o")
                            nc.vector.tensor_copy(o, po)
                            nc.sync.dma_start(
                                out=pre_d[qrow : qrow + P, os_], in_=o
                            )

                # pipeline: B0 B1 C0 B2 C1 B3 C2 C3
                phase_b(0)
                phase_b(1)
                phase_c(0)
                phase_b(2)
                phase_c(1)
                phase_b(3)
                phase_c(2)
                phase_c(3)

    _split_excess_waits(nc)
    return nc


def _get_nc():
    if "nc" not in _CACHED:
        _CACHED["nc"] = _build()
    return _CACHED["nc"]


def _split8(a):
    hi = a.astype(NP8)
    lo = (a - hi.astype(np.float32)).astype(NP8)
    return hi, lo


def kernel(x, Wq, bq, Wk, bk, Wv, bv):
    x = np.asarray(x, dtype=np.float32)
    Wq32 = np.asarray(Wq, np.float32)
    Wk32 = np.asarray(Wk, np.float32)
    bq32 = np.asarray(bq, np.float32)
    bk32 = np.asarray(bk, np.float32)
    bv32 = np.asarray(bv, np.float32)
    # weight fusion: gT = Wk Wq^T (so scores = x G x^T); w = Wk bq; c0 = bq.bk
    gh, gl = _split8(np.ascontiguousarray(Wk32 @ Wq32.T) * np.float32(GS))
    wvh, wvl = _split8(
        np.ascontiguousarray(np.asarray(Wv, np.float32)) * np.float32(VS)
    )
    w8 = (
        np.ascontiguousarray((Wk32 @ bq32).reshape(DC, P).T) * np.float32(WS)
    ).astype(NP8)
    c0s = np.full((P, 1), float(SCALE) * float(bq32 @ bk32) - CSH, np.float32)

    in_maps = []
    for core in range(NCORES):
        b, h = core // 2, core % 2
        # own j rows first (j order is internal; q order is undone on gather)
        xb = np.roll(x[b], -h * JROWS, axis=0) if h else x[b]
        xT = np.ascontiguousarray(xb.T)  # [D, S] f32
        xTh, xTl = _split8(xT)
        in_maps.append(
            {
                "xh": xTh, "xl": xTl,
                "gh": gh, "gl": gl,
                "wvh": wvh, "wvl": wvl,
                "w": w8, "c0s": c0s,
            }
        )

    res = run_bass_kernel_spmd(_get_nc(), in_maps, list(range(NCORES)))
    out = np.empty((B, S, D), np.float32)
    inv_vs = np.float32(1.0 / VS)
    for b in range(B):
        r0, r1 = res.results[2 * b], res.results[2 * b + 1]
        pre = r0["pre"].astype(np.float32) + np.roll(
            r1["pre"].astype(np.float32), JROWS, axis=0
        )
        den = r0["den"] + np.roll(r1["den"], JROWS, axis=0)
        out[b] = pre / den * inv_vs + bv32[None, :]
    return out
